# revision 42
# baseline (speedup 1.0000x reference)
"""ALIGNN (12x EdgeGatedGraphConv, H=256) on 8 TRN2 NeuronCores.

Sharding: nodes contiguously partitioned over cores; edges partitioned by dst
owner into 4 node-windows x 18 tiles (ECg=9216 slots/core), with a 2D-balanced
edge->tile assignment so each line-graph scatter window's per-gather-half load
fits 2 tiles. Line-graph gather table split in two position halves (crystal
windows {0,1} vs {2,3}); each half is exported + AllGather-ed as soon as the
covering phase-4 chunks finish, and line edge tiles are sorted half-0-first so
AG(h1) hides behind h0 tiles. Feature-major bf16 activations. Source-row
gathers via indirect DMA from row-major bf16 tables. e_dst expand + segment
sum via 0/1 indicator matmuls; m transposed to feature-major on PE. BN stats
via masked rank-1 matmuls + one small AllReduce per conv; BN affine + SiLU
fused on ScalarE.
"""
import numpy as np

H = 256
H2 = 512
NCORES = 8
P = 128
CHUNK = 1024
TPC = CHUNK // P  # tiles per chunk
N_NODES = 4096
N_EDGES = 49152
N_TRIPLETS = 262144
BN_EPS = 1e-5
F32 = np.float32
TILES_PER_WIN = 18  # crystal: diluted edge tiles per node-window
CAP = 2 * P         # per-(line-window, half) triplet budget -> 2 tiles
GATHER_BATCH = 1    # tiles per indirect gather (1 or 2)


# --------------------------------------------------------------------------
# host-side prep
# --------------------------------------------------------------------------

def _balance_buckets(src, dst, n_nodes, f0, f1):
    """Crystal graph: bucket edges by (dst owner, dst window); within each
    bucket assign edges to TILES_PER_WIN tiles of 128 slots, keeping each
    tile's (f0,f1) sums <= CAP where possible (controls line-graph padding).
    Returns per-core list of per-bucket edge-id lists (one list per tile)."""
    NR = n_nodes // NCORES
    NW = NR // P
    owner = dst // NR
    out = {}
    for c in range(NCORES):
        sel = np.where(owner == c)[0]
        w = (dst[sel] - c * NR) // P
        for wi in range(NW):
            ids = sel[w == wi]
            o = np.argsort(-(f0[ids] + f1[ids]), kind="stable")
            ids = ids[o]
            nt = TILES_PER_WIN
            l0 = np.zeros(nt); l1 = np.zeros(nt)
            cnt = np.zeros(nt, np.int64)
            tiles = [[] for _ in range(nt)]
            for e in ids:
                a, b = f0[e], f1[e]
                ok = (cnt < P) & (l0 + a <= CAP) & (l1 + b <= CAP)
                if ok.any():
                    cand = np.where(ok)[0]
                    k = cand[np.argmin(np.maximum(l0[cand] + a,
                                                  l1[cand] + b))]
                else:
                    cand = np.where(cnt < P)[0]
                    k = cand[np.argmin(np.maximum(l0[cand] + a,
                                                  l1[cand] + b))]
                tiles[k].append(e)
                l0[k] += a; l1[k] += b; cnt[k] += 1
            out[(c, wi)] = tiles
    return out


def _prep_crystal(src, dst, f0, f1):
    """Build crystal-graph layout with fixed TILES_PER_WIN tiles/window."""
    NR = N_NODES // NCORES
    NW = NR // P
    EC = NW * TILES_PER_WIN * P
    n_tiles = EC // P
    assn = _balance_buckets(src, dst, N_NODES, f0, f1)
    g = {"EC": EC, "NR": NR, "NW": NW, "NH": 1, "n_tiles": n_tiles,
         "tile_win": np.zeros(n_tiles, np.int64),
         "tile_half": np.zeros(n_tiles, np.int64),
         "tile_run_start": np.zeros(n_tiles, bool),
         "tile_run_end": np.zeros(n_tiles, bool),
         "tile_tail": np.zeros(n_tiles, bool),
         "idx": np.zeros((NCORES, EC), np.int32),
         "mask": np.zeros((NCORES, EC), F32),
         "ind_col": -np.ones((NCORES, EC), np.int64),
         "edge_id": -np.ones((NCORES, EC), np.int64)}
    t = 0
    for wi in range(NW):
        for k in range(TILES_PER_WIN):
            for c in range(NCORES):
                ids = np.array(assn[(c, wi)][k], np.int64)
                sl = slice(t * P, t * P + len(ids))
                g["idx"][c, sl] = src[ids].astype(np.int32)
                g["mask"][c, sl] = 1.0
                g["ind_col"][c, sl] = (dst[ids] - c * NR) - wi * P
                g["edge_id"][c, sl] = ids
            g["tile_win"][t] = wi
            g["tile_run_start"][t] = k == 0
            g["tile_run_end"][t] = k == TILES_PER_WIN - 1
            t += 1
    assert t == n_tiles and EC % CHUNK == 0
    return g


def _prep_line(src, dst, table_rows, pos_half, chunk=CHUNK):
    """Line graph: partition triplets by dst-slot owner; bucket by
    (gather half = src-slot position-half, dst window); pad per (half,window)
    to core-uniform tiles. idx values are relative to the half table."""
    E = len(src)
    NR = table_rows // NCORES
    NW = NR // P
    owner = dst // NR
    hs = pos_half // 2
    so, sp = src // pos_half, src % pos_half
    shalf = (sp >= hs).astype(np.int64)
    sidx = (so * hs + (sp % hs)).astype(np.int32)
    NH = 2
    buckets = {}
    for c in range(NCORES):
        sel = np.where(owner == c)[0]
        w = (dst[sel] - c * NR) // P
        hh = shalf[sel]
        for h in range(NH):
            for wi in range(NW):
                buckets[(c, h, wi)] = sel[(hh == h) & (w == wi)]
    T_hw = np.zeros((NH, NW), np.int64)
    for h in range(NH):
        for wi in range(NW):
            mx = max(len(buckets[(c, h, wi)]) for c in range(NCORES))
            T_hw[h, wi] = max(1, -(-mx // P))
    S_h = [int(T_hw[h].sum() * P) for h in range(NH)]
    S_h_pad = [-(-s // chunk) * chunk for s in S_h]
    EC = sum(S_h_pad)
    n_tiles = EC // P
    g = {"EC": EC, "NR": NR, "NW": NW, "NH": NH, "n_tiles": n_tiles,
         "tile_win": np.zeros(n_tiles, np.int64),
         "tile_half": np.zeros(n_tiles, np.int64),
         "tile_run_start": np.zeros(n_tiles, bool),
         "tile_run_end": np.zeros(n_tiles, bool),
         "tile_tail": np.zeros(n_tiles, bool),
         "idx": np.zeros((NCORES, EC), np.int32),
         "mask": np.zeros((NCORES, EC), F32),
         "ind_col": -np.ones((NCORES, EC), np.int64),
         "edge_id": -np.ones((NCORES, EC), np.int64)}
    t = pos = 0
    for h in range(NH):
        for wi in range(NW):
            nt = int(T_hw[h, wi])
            for c in range(NCORES):
                ids = buckets[(c, h, wi)]
                sl = slice(pos, pos + len(ids))
                g["idx"][c, sl] = sidx[ids]
                g["mask"][c, sl] = 1.0
                g["ind_col"][c, sl] = (dst[ids] - c * NR) - wi * P
                g["edge_id"][c, sl] = ids
            g["tile_win"][t:t + nt] = wi
            g["tile_half"][t:t + nt] = h
            g["tile_run_start"][t] = True
            g["tile_run_end"][t + nt - 1] = True
            t += nt
            pos += nt * P
        tail = (S_h_pad[h] - S_h[h]) // P
        for _ in range(tail):
            g["tile_win"][t] = 0
            g["tile_half"][t] = h
            g["tile_run_start"][t] = True
            g["tile_run_end"][t] = True
            g["tile_tail"][t] = True
            t += 1
            pos += P
    assert t == n_tiles and pos == EC
    return g


def _indicators(g, bdt):
    C, T = NCORES, g["n_tiles"]
    ind = np.zeros((C, T, P, P), F32)
    for c in range(C):
        cols = g["ind_col"][c]
        for t in range(T):
            cl = cols[t * P:(t + 1) * P]
            e = np.where(cl >= 0)[0]
            ind[c, t, e, cl[e]] = 1.0
    indT = ind.transpose(0, 1, 3, 2)
    # slab layout [T//TPC, 128, TPC, 128] (partition-major per chunk)
    def slab(a):
        return np.ascontiguousarray(
            a.reshape(C, T // TPC, TPC, P, P).transpose(0, 1, 3, 2, 4)).astype(bdt)
    return slab(ind), slab(indT)


def _cols(a, dt):  # [C, EC] -> [C, 128, T] (column t = tile t values)
    C, EC = a.shape
    return np.ascontiguousarray(
        a.reshape(C, EC // P, P).transpose(0, 2, 1)).astype(dt)


def _fm(a, bdt):  # [N, 256] -> [128, 2, N] feature-major
    return np.ascontiguousarray(a.T.reshape(2, P, -1).transpose(1, 0, 2)).astype(bdt)


def prep_all(inputs, bdt):
    src = np.asarray(inputs["src"]); dst = np.asarray(inputs["dst"])
    lsrc = np.asarray(inputs["lsrc"]); ldst = np.asarray(inputs["ldst"])
    # gather half of triplet t = crystal window-group of lsrc[t]'s dst node
    win_of_edge = (dst % (N_NODES // NCORES)) // P          # 0..3
    h_t = (win_of_edge[lsrc] >= 2).astype(np.int64)         # triplet half
    f0 = np.bincount(ldst[h_t == 0], minlength=N_EDGES)     # per-edge loads
    f1 = np.bincount(ldst[h_t == 1], minlength=N_EDGES)
    gg = _prep_crystal(src, dst, f0, f1)
    ECg = gg["EC"]
    BHALF = ECg // 2
    slot = np.zeros(N_EDGES, np.int64)
    for c in range(NCORES):
        eids = gg["edge_id"][c]
        r = eids >= 0
        slot[eids[r]] = c * ECg + np.where(r)[0]
    TABL = NCORES * ECg
    gl = _prep_line(slot[lsrc], slot[ldst], TABL, ECg)
    meta = {"gg": gg, "gl": gl, "TABL": TABL}

    x = np.asarray(inputs["x"]); y = np.asarray(inputs["y"])
    z = np.asarray(inputs["z"])
    W = np.asarray(inputs["W"]); b = np.asarray(inputs["b"])
    bg = np.asarray(inputs["bn_gamma"]); bb = np.asarray(inputs["bn_beta"])
    ECl = gl["EC"]

    ind_g, indT_g = _indicators(gg, bdt)
    ind_l, indT_l = _indicators(gl, bdt)
    mask_g = _cols(gg["mask"], bdt)
    mask_l = _cols(gl["mask"], bdt)
    idx_g = _cols(gg["idx"].astype(np.int32), np.int32)
    idx_l = _cols(gl["idx"].astype(np.int32), np.int32)

    # per-core feature tensors
    xT = np.zeros((NCORES, P, 2, 512), bdt)
    yT = np.zeros((NCORES, P, 2, ECg), bdt)
    zT = np.zeros((NCORES, P, 2, ECl), bdt)
    for c in range(NCORES):
        xT[c] = _fm(x[c * 512:(c + 1) * 512], bdt)
        eg = gg["edge_id"][c]; r = eg >= 0
        yr = np.zeros((ECg, H), F32); yr[r] = y[eg[r]]
        yT[c] = _fm(yr, bdt)
        el = gl["edge_id"][c]; rl = el >= 0
        zr = np.zeros((ECl, H), F32); zr[rl] = z[el[rl]]
        zT[c] = _fm(zr, bdt)
    mbcg = np.repeat(gg["mask"][:, None, :], P, axis=1).astype(bdt)  # [C,128,ECg]
    xb = x.astype(bdt).astype(F32)
    W0b = W[0, 0].astype(bdt).astype(F32)
    W4b = W[0, 4].astype(bdt).astype(F32)
    b4b = b[0, 4].astype(F32)
    bm0 = (b[0, 0] + b[0, 1] + b[0, 2]).astype(F32)
    xtab0 = np.concatenate([xb @ W0b + bm0, xb @ W4b + b4b], 1).astype(bdt)

    # weights: w04 [12,128,2,512]; w1/w2 [12,128,2,256]; w3 [12,128,2,2,128]
    w04 = np.concatenate([W[:, 0], W[:, 4]], axis=2)  # [12,256,512]
    w04 = w04.reshape(12, 2, P, H2).transpose(0, 2, 1, 3).astype(bdt).copy()
    w1 = W[:, 1].reshape(12, 2, P, H).transpose(0, 2, 1, 3).astype(bdt).copy()
    w2 = W[:, 2].reshape(12, 2, P, H).transpose(0, 2, 1, 3).astype(bdt).copy()
    w3 = W[:, 3].reshape(12, 2, P, 2, P).transpose(0, 2, 1, 3, 4).astype(bdt).copy()
    bmr = (b[:, 0] + b[:, 1] + b[:, 2]).reshape(12, 1, H)
    b4r = b[:, 4].reshape(12, 1, H)
    bmb4 = np.concatenate([bmr, b4r], axis=2).astype(bdt)  # [12,1,512]
    b3r = b[:, 3].reshape(12, 1, 2, P).astype(bdt)
    # gamma/beta packed [12,128,4]: cols (node h0, node h1, edge h0, edge h1)
    gbg = np.stack([bg[:, 0, :P], bg[:, 0, P:], bg[:, 1, :P], bg[:, 1, P:]], -1)
    gbb = np.stack([bb[:, 0, :P], bb[:, 0, P:], bb[:, 1, :P], bb[:, 1, P:]], -1)

    # pad corrections for unmasked node stats of line convs: pad slots hold
    # exactly b3(bf16) after the node update; subtract their sums post-AR.
    npads = float(NCORES * ECg - N_EDGES)
    b3bf = b[:, 3].astype(bdt).astype(F32)               # [12, 256]
    scorr = np.zeros((12, P, 2), F32)
    qcorr = np.zeros((12, P, 2), F32)
    for li in (1, 3, 5, 7):
        v = b3bf[li].reshape(2, P).T                     # [P, 2]
        scorr[li] = npads * v
        qcorr[li] = npads * v * v

    shared = {"xtab0": xtab0, "w04": w04, "w1": w1, "w2": w2, "w3": w3,
              "bmb4": bmb4, "b3r": b3r,
              "gbg": gbg.astype(F32), "gbb": gbb.astype(F32),
              "scorr": scorr, "qcorr": qcorr}
    in_maps = []
    for c in range(NCORES):
        m = dict(shared)
        m.update({"xT": xT[c], "yT": yT[c], "zT": zT[c],
                  "idxg": idx_g[c], "idxl": idx_l[c],
                  "indg": ind_g[c], "indTg": indT_g[c], "maskg": mask_g[c],
                  "indl": ind_l[c], "indTl": indT_l[c], "maskl": mask_l[c],
                  "mbcg": mbcg[c]})
        in_maps.append(m)
    return meta, in_maps


# --------------------------------------------------------------------------
# bass kernel
# --------------------------------------------------------------------------

def build_bass(meta):
    import concourse.bass as bass
    import concourse.bacc as bacc
    import concourse.tile as tile
    from concourse import mybir
    from concourse.masks import make_identity

    BF = mybir.dt.bfloat16
    FP = mybir.dt.float32
    I32 = mybir.dt.int32
    AL = mybir.AluOpType
    AF = mybir.ActivationFunctionType
    AX = mybir.AxisListType
    RG = [list(range(NCORES))]

    gg, gl = meta["gg"], meta["gl"]
    ECg, ECl = gg["EC"], gl["EC"]
    NWg, NWl = gg["NW"], gl["NW"]
    Tg, Tl = gg["n_tiles"], gl["n_tiles"]
    TABL = meta["TABL"]
    BHALF = ECg // 2          # line gather-table half boundary (positions)
    JH = BHALF // P           # export j-tiles per half
    CK_H0 = -(-BHALF // CHUNK) - 1   # phase-4 chunk covering half 0
    CK_H1 = ECg // CHUNK - 1

    nc = bacc.Bacc("TRN2", target_bir_lowering=False, debug=False,
                   num_devices=NCORES)

    def din(name, shape, dt):
        return nc.dram_tensor(name, shape, dt, kind="ExternalInput").ap()

    xT_i = din("xT", [P, 2, 512], BF)
    yT_i = din("yT", [P, 2, ECg], BF)
    zT_i = din("zT", [P, 2, ECl], BF)
    xtab0_i = din("xtab0", [N_NODES, H2], BF)
    idxg_i = din("idxg", [P, Tg], I32)
    idxl_i = din("idxl", [P, Tl], I32)
    indg_i = din("indg", [Tg // TPC, P, TPC, P], BF)
    indTg_i = din("indTg", [Tg // TPC, P, TPC, P], BF)
    maskg_i = din("maskg", [P, Tg], BF)
    indl_i = din("indl", [Tl // TPC, P, TPC, P], BF)
    indTl_i = din("indTl", [Tl // TPC, P, TPC, P], BF)
    maskl_i = din("maskl", [P, Tl], BF)
    mbcg_i = din("mbcg", [P, ECg], BF)
    w04_i = din("w04", [12, P, 2, H2], BF)
    w1_i = din("w1", [12, P, 2, H], BF)
    w2_i = din("w2", [12, P, 2, H], BF)
    w3_i = din("w3", [12, P, 2, 2, P], BF)
    bmb4_i = din("bmb4", [12, 1, H2], BF)
    b3r_i = din("b3r", [12, 1, 2, P], BF)
    gbg_i = din("gbg", [12, P, 4], FP)
    gbb_i = din("gbb", [12, P, 4], FP)
    scorr_i = din("scorr", [12, P, 2], FP)
    qcorr_i = din("qcorr", [12, P, 2], FP)
    out_o = nc.dram_tensor("out", [512, H], FP, kind="ExternalOutput").ap()

    with tile.TileContext(nc) as tc:
        with (
            tc.tile_pool(name="dram", bufs=1, space="DRAM") as dram,
            tc.tile_pool(name="dram2", bufs=2, space="DRAM") as dram2,
            tc.tile_pool(name="persist", bufs=1) as persist,
            tc.tile_pool(name="wpool", bufs=2) as wpool,
            tc.tile_pool(name="chk", bufs=2) as chk,
            tc.tile_pool(name="work", bufs=3) as work,
            tc.tile_pool(name="small", bufs=2) as small,
            tc.tile_pool(name="pe", bufs=2, space="PSUM") as pe_p,
            tc.tile_pool(name="pm", bufs=2, space="PSUM") as pm_p,
            tc.tile_pool(name="pn", bufs=2, space="PSUM") as pn_p,
            tc.tile_pool(name="ptb", bufs=1, space="PSUM") as ptb_p,
            tc.tile_pool(name="ps", bufs=1, space="PSUM") as ps_p,
        ):
            z_state = dram.tile([P, 2, ECl], BF)

            m_rows = dram.tile([ECg, H2], BF)
            x_rows = dram.tile([512, H2], BF)

            yT = persist.tile([P, 2, ECg], BF)
            xT = persist.tile([P, 2, 512], BF)
            mbcg = persist.tile([P, ECg], BF)
            edst_l = persist.tile([P, NWl, H], BF)
            edst_g = persist.tile([P, NWg, H], BF)
            idxg = persist.tile([P, Tg], I32)
            idxl = persist.tile([P, Tl], I32)
            maskg_t = persist.tile([P, Tg], BF)
            maskl_t = persist.tile([P, Tl], BF)
            ident = persist.tile([P, P], BF)
            identF = persist.tile([P, P], FP)
            ones1 = persist.tile([1, H2], BF)
            onesP = persist.tile([P, 1], BF)
            epsb = persist.tile([P, 1], FP)

            nc.sync.dma_start(out=yT[:], in_=yT_i[:])
            nc.sync.dma_start(out=xT[:], in_=xT_i[:])
            nc.sync.dma_start(out=mbcg[:], in_=mbcg_i[:])
            nc.sync.dma_start(out=idxg[:], in_=idxg_i[:])
            nc.sync.dma_start(out=idxl[:], in_=idxl_i[:])
            nc.sync.dma_start(out=maskg_t[:], in_=maskg_i[:])
            nc.sync.dma_start(out=maskl_t[:], in_=maskl_i[:])
            make_identity(nc, ident[:])
            make_identity(nc, identF[:])
            nc.gpsimd.memset(ones1[:], 1.0)
            nc.gpsimd.memset(onesP[:], 1.0)
            nc.gpsimd.memset(epsb[:], 1e-6)

            def emit_conv(i, line, tabs, post_3b=None, post_p4=None):
                g = gl if line else gg
                T, NW, EC = g["n_tiles"], g["NW"], g["EC"]
                NH = g["NH"]
                n_rn = N_EDGES if line else N_NODES
                n_re = N_TRIPLETS if line else N_EDGES
                nfT = yT if line else xT
                edst = edst_l if line else edst_g
                idx = idxl if line else idxg
                ind_i, indT_i, mask_i = ((indl_i, indTl_i, maskl_i) if line
                                         else (indg_i, indTg_i, maskg_i))

                w1 = wpool.tile([P, 2, H], BF, tag="w1")
                w2 = wpool.tile([P, 2, H], BF, tag="w2")
                w3 = wpool.tile([P, 2, 2, P], BF, tag="w3")
                b3r = wpool.tile([1, 2, P], BF, tag="b3r")
                gbg = wpool.tile([P, 4], FP, tag="gbg")
                gbb = wpool.tile([P, 4], FP, tag="gbb")
                invn = wpool.tile([P, 4], FP, tag="invn")
                scor = wpool.tile([P, 2], FP, tag="scor")
                qcor = wpool.tile([P, 2], FP, tag="qcor")
                for tl, src_t in ((w1, w1_i), (w2, w2_i),
                                  (w3, w3_i),
                                  (b3r, b3r_i), (gbg, gbg_i), (gbb, gbb_i),
                                  (scor, scorr_i), (qcor, qcorr_i)):
                    nc.gpsimd.dma_start(out=tl[:], in_=src_t[i])
                nc.vector.memset(invn[:, 0:2], 1.0 / n_rn)
                nc.vector.memset(invn[:, 2:4], 1.0 / n_re)

                # ---- node phase: e_dst windows (bias folded into tables) ----
                for w in range(NW):
                    pw = pe_p.tile([P, H2], FP, tag="pe")
                    sl = slice(w * P, (w + 1) * P)
                    for kh in range(2):
                        nc.tensor.matmul(pw[:, :H], nfT[:, kh, sl], w1[:, kh, :],
                                         start=kh == 0, stop=kh == 1,
                                         skip_group_check=True)
                    nc.scalar.activation(edst[:, w, :], pw[:, :H], AF.Copy)

                # ---- edge phase (3a interleaved per completed window) ----
                m_scr = dram2.tile([ECl, H], BF, tag="mscr",
                                   name=f"mscr_{i}")
                naccl_d = None
                if line:
                    naccl_d = dram2.tile([NWl, P, 4 * P], BF, tag="naccl",
                                         name=f"naccl_{i}")
                pstat = ps_p.tile([1, H2], FP, tag="ps")
                pnode = None
                mask_t = maskl_t if line else maskg_t
                scs = small.tile([P, NW, 4], FP, name=f"scs_{i}", tag="scs")

                def emit_3a(w, pn_ap):
                    """Node update for window w; pn_ap = [P, 512] accumulated
                    (sum_sigma_h || sum_sigma), PSUM or SBUF."""
                    wsl = slice(w * P, (w + 1) * P)
                    den = small.tile([P, H], FP, tag="den")
                    hrow = small.tile([P, H], FP, tag="hrow")
                    nc.vector.tensor_scalar_add(den[:], pn_ap[:, H:], 1e-6)
                    nc.vector.reciprocal(den[:], den[:])
                    nc.vector.tensor_tensor(out=hrow[:], in0=pn_ap[:, :H],
                                            in1=den[:], op=AL.mult)
                    pnu = pe_p.tile([P, H2], FP, tag="pe")
                    for b_ in range(2):
                        bsl = slice(b_ * P, (b_ + 1) * P)
                        for a_ in range(2):
                            nc.tensor.matmul(pnu[:, bsl], w3[:, a_, b_, :],
                                             nfT[:, a_, wsl], start=a_ == 0,
                                             stop=False, skip_group_check=True)
                        nc.tensor.matmul(pnu[:, bsl], b3r[:, b_, :],
                                         ones1[:, :P], start=False, stop=False,
                                         skip_group_check=True)
                        # h^T accumulated straight into the psum group
                        nc.tensor.matmul(pnu[:, bsl], hrow[:, bsl], identF[:],
                                         is_transpose=True, start=False,
                                         stop=b_ == 1, skip_group_check=True)
                    # store nodupd + BN stats (unmasked; host pad-corrections)
                    nup = edst[:, w, :]
                    nc.scalar.activation(nup, pnu[:, :H], AF.Copy)
                    tq = small.tile([P, 2, P], BF, tag="tq")
                    nup2 = nup.rearrange("p (a b) -> p a b", a=2)
                    nc.scalar.square(tq[:], nup2)
                    nc.vector.tensor_reduce(scs[:, w, 0:2], nup2,
                                            axis=AX.X, op=AL.add)
                    nc.vector.tensor_reduce(scs[:, w, 2:4], tq[:],
                                            axis=AX.X, op=AL.add)

                for ck in range(EC // CHUNK):
                    t0 = ck * TPC
                    csl = slice(ck * CHUNK, (ck + 1) * CHUNK)
                    if line:
                        zt = chk.tile([P, 2, CHUNK], BF, tag="zt")
                        zsrc = zT_i if i == 1 else z_state
                        nc.sync.dma_start(out=zt[:], in_=zsrc[:, :, csl])
                    inds = chk.tile([P, TPC, P], BF, tag="inds")
                    indTs = chk.tile([P, TPC, P], BF, tag="indTs")
                    nc.sync.dma_start(out=inds[:], in_=ind_i[ck])
                    nc.sync.dma_start(out=indTs[:], in_=indT_i[ck])

                    gath = None
                    sbm = None
                    for tt in range(TPC):
                        t = t0 + tt
                        w = int(g["tile_win"][t])
                        hh = int(g["tile_half"][t])
                        tail = bool(g["tile_tail"][t])
                        # gathers batched in pairs when same table half
                        gq = tt % 2
                        if gq == 0:
                            pair = (GATHER_BATCH == 2
                                    and tt + 1 < TPC
                                    and not tail
                                    and not bool(g["tile_tail"][t + 1])
                                    and int(g["tile_half"][t + 1]) == hh)
                            gath = work.tile([P, 2, H2], BF, tag="gath", bufs=4)
                            if pair:
                                nc.gpsimd.indirect_dma_start(
                                    out=gath[:], out_offset=None, in_=tabs[hh],
                                    in_offset=bass.IndirectOffsetOnAxis(
                                        ap=idx[:, t:t + 2], axis=0))
                        if not (gq == 0 and pair):
                            if tail:
                                nc.vector.memset(gath[:, gq, :], 0.0)
                            elif gq == 1 and not pair:
                                nc.gpsimd.indirect_dma_start(
                                    out=gath[:, 1, :], out_offset=None,
                                    in_=tabs[hh],
                                    in_offset=bass.IndirectOffsetOnAxis(
                                        ap=idx[:, t:t + 1], axis=0))
                            elif gq == 0:
                                nc.gpsimd.indirect_dma_start(
                                    out=gath[:, 0, :], out_offset=None,
                                    in_=tabs[hh],
                                    in_offset=bass.IndirectOffsetOnAxis(
                                        ap=idx[:, t:t + 1], axis=0))
                        ga = gath[:, gq, :]
                        pm = pm_p.tile([P, H], FP, tag="pm")
                        nc.tensor.matmul(pm[:], indTs[:, tt, :],
                                         edst[:, w, :], start=True, stop=False)
                        for kh in range(2):
                            ef = (zt[:, kh, tt * P:(tt + 1) * P] if line
                                  else yT[:, kh, ck * CHUNK + tt * P:
                                          ck * CHUNK + (tt + 1) * P])
                            nc.tensor.matmul(pm[:], ef, w2[:, kh, :],
                                             start=False, stop=kh == 1)
                        if gq == 0:
                            sbm = work.tile([P, 2, H2], BF, tag="sbm", bufs=3)
                        sbw = work.tile([P, H2], BF, tag="sbw", bufs=6)
                        # m = e_src(gathered, +bias) + e_dst_expand + egate
                        nc.vector.tensor_tensor(out=sbm[:, gq, :H],
                                                in0=ga[:, :H],
                                                in1=pm[:], op=AL.add)
                        nc.scalar.activation(sbw[:, H:], sbm[:, gq, :H],
                                             AF.Sigmoid)
                        nc.scalar.square(sbm[:, gq, H:], sbm[:, gq, :H])
                        nc.vector.tensor_tensor(out=sbw[:, :H], in0=sbw[:, H:],
                                                in1=ga[:, H:], op=AL.mult)
                        if g["tile_run_start"][t]:
                            pnode = pn_p.tile([P, 4 * P], FP, tag="pn")
                        nc.tensor.matmul(pnode[:], inds[:, tt, :], sbw[:],
                                         start=bool(g["tile_run_start"][t]),
                                         stop=bool(g["tile_run_end"][t]),
                                         skip_group_check=True)
                        nc.tensor.matmul(pstat[:], mask_t[:, t:t + 1],
                                         sbm[:, gq, :],
                                         start=t == 0, stop=t == T - 1,
                                         skip_group_check=True)
                        # m rows stored slot-major, 2 tiles per DMA
                        if gq == 1:
                            nc.sync.dma_start(
                                out=m_scr[(t - 1) * P:(t + 1) * P, :].rearrange(
                                    "(b p) f -> p b f", b=2),
                                in_=sbm[:, :, :H])
                        if g["tile_run_end"][t] and not tail:
                            if line and hh == 0:
                                nae = small.tile([P, 4 * P], BF, tag="nae")
                                nc.vector.tensor_copy(out=nae[:], in_=pnode[:])
                                nc.sync.dma_start(out=naccl_d[w], in_=nae[:])
                            elif line:
                                na0 = small.tile([P, 4 * P], BF, tag="naccld")
                                nc.sync.dma_start(out=na0[:], in_=naccl_d[w])
                                nacc = small.tile([P, 4 * P], FP, tag="naccs")
                                nc.vector.tensor_tensor(out=nacc[:], in0=na0[:],
                                                        in1=pnode[:], op=AL.add)
                                emit_3a(w, nacc)
                            else:
                                emit_3a(w, pnode)

                # ---- stats allreduce ----
                nst = small.tile([P, 4], FP, tag="nst")
                for j in range(4):
                    nc.vector.tensor_reduce(nst[:, j:j + 1], scs[:, :, j],
                                            axis=AX.X, op=AL.add)
                est = small.tile([1, H2], FP, tag="est")
                nc.vector.tensor_copy(out=est[:], in_=pstat[:])
                stb_i = dram2.tile([1024], FP, tag="stbi")
                stb_o = dram2.tile([1024], FP, tag="stbo")
                nc.sync.dma_start(
                    out=stb_i[0:H2].rearrange("(c p) -> p c", p=P), in_=nst[:])
                nc.sync.dma_start(out=stb_i[H2:].rearrange("(a b) -> a b", a=1), in_=est[:])
                nc.gpsimd.collective_compute(
                    "AllReduce", AL.add, replica_groups=RG,
                    ins=[stb_i[:].opt()], outs=[stb_o[:].opt()])
                SS = small.tile([P, 4], FP, tag="SS")
                QQ = small.tile([P, 4], FP, tag="QQ")
                nc.sync.dma_start(out=SS[:, 0:2],
                                  in_=stb_o[0:256].rearrange("(c p) -> p c", p=P))
                nc.sync.dma_start(out=QQ[:, 0:2],
                                  in_=stb_o[256:512].rearrange("(c p) -> p c", p=P))
                nc.sync.dma_start(out=SS[:, 2:4],
                                  in_=stb_o[512:768].rearrange("(c p) -> p c", p=P))
                nc.sync.dma_start(out=QQ[:, 2:4],
                                  in_=stb_o[768:1024].rearrange("(c p) -> p c", p=P))
                # pad-slot corrections for the unmasked node stats
                nc.vector.tensor_tensor(out=SS[:, 0:2], in0=SS[:, 0:2],
                                        in1=scor[:], op=AL.subtract)
                nc.vector.tensor_tensor(out=QQ[:, 0:2], in0=QQ[:, 0:2],
                                        in1=qcor[:], op=AL.subtract)
                mu = small.tile([P, 4], FP, tag="mu")
                var = small.tile([P, 4], FP, tag="var")
                sc4 = small.tile([P, 4], FP, tag="sc4")
                sh4 = small.tile([P, 4], FP, tag="sh4")
                nc.vector.tensor_tensor(out=mu[:], in0=SS[:], in1=invn[:], op=AL.mult)
                nc.vector.tensor_tensor(out=var[:], in0=QQ[:], in1=invn[:], op=AL.mult)
                nc.vector.tensor_tensor(out=sc4[:], in0=mu[:], in1=mu[:], op=AL.mult)
                nc.vector.tensor_tensor(out=var[:], in0=var[:], in1=sc4[:],
                                        op=AL.subtract)
                nc.vector.tensor_scalar_add(var[:], var[:], BN_EPS)
                nc.scalar.sqrt(var[:], var[:])
                nc.vector.reciprocal(var[:], var[:])
                nc.vector.tensor_tensor(out=sc4[:], in0=var[:], in1=gbg[:], op=AL.mult)
                nc.vector.tensor_tensor(out=mu[:], in0=mu[:], in1=sc4[:], op=AL.mult)
                nc.vector.tensor_tensor(out=sh4[:], in0=gbb[:], in1=mu[:],
                                        op=AL.subtract)

                # ---- phase 3b: node update ----
                for w in range(NW):
                    wsl = slice(w * P, (w + 1) * P)
                    t3 = small.tile([P, 2, P], BF, tag="t3")
                    for hf in range(2):
                        nc.scalar.activation(t3[:, hf, :],
                                             edst[:, w, hf * P:(hf + 1) * P],
                                             AF.Silu, bias=sh4[:, hf:hf + 1],
                                             scale=sc4[:, hf:hf + 1])
                        if line:
                            nc.vector.tensor_tensor(out=t3[:, hf, :],
                                                    in0=t3[:, hf, :],
                                                    in1=mbcg[:, wsl], op=AL.mult)
                    nc.vector.tensor_tensor(out=nfT[:, :, wsl],
                                            in0=nfT[:, :, wsl], in1=t3[:],
                                            op=AL.add)
                if post_3b is not None:
                    post_3b()

                # ---- phase 4: edge update (2-chunk units, in-place) ----
                NCK = EC // CHUNK
                ck = 0
                while ck < NCK:
                    u = min(2, NCK - ck)
                    CW = u * CHUNK
                    csl = slice(ck * CHUNK, ck * CHUNK + CW)
                    mt = chk.tile([P, 2, 2 * CHUNK], BF, tag="mt")
                    for hf in range(2):
                        nc.sync.dma_start_transpose(
                            out=mt[:, hf, :CW],
                            in_=m_scr[csl, hf * P:(hf + 1) * P])
                    if line:
                        zt2 = chk.tile([P, 2, 2 * CHUNK], BF, tag="zt")
                        zsrc = zT_i if i == 1 else z_state
                        nc.sync.dma_start(out=zt2[:, :, :CW],
                                          in_=zsrc[:, :, csl])
                    for hf in range(2):
                        nc.scalar.activation(mt[:, hf, :CW], mt[:, hf, :CW],
                                             AF.Silu,
                                             bias=sh4[:, 2 + hf:3 + hf],
                                             scale=sc4[:, 2 + hf:3 + hf])
                    if line:
                        for hf in range(2):
                            nc.vector.tensor_tensor(out=mt[:, hf, :CW],
                                                    in0=mt[:, hf, :CW],
                                                    in1=zt2[:, hf, :CW],
                                                    op=AL.add)
                        nc.sync.dma_start(out=z_state[:, :, csl],
                                          in_=mt[:, :, :CW])
                    else:
                        for hf in range(2):
                            nc.vector.tensor_tensor(out=mt[:, hf, :CW],
                                                    in0=mt[:, hf, :CW],
                                                    in1=mbcg[:, csl], op=AL.mult)
                            nc.vector.tensor_tensor(out=yT[:, hf, csl],
                                                    in0=yT[:, hf, csl],
                                                    in1=mt[:, hf, :CW],
                                                    op=AL.add)
                    if post_p4 is not None:
                        for c2 in range(ck, ck + u):
                            post_p4(c2)
                    ck += u

            # ---- table exports ----
            _exp_w = {}

            def export_prep(consumer, name):
                w04c = wpool.tile([P, 2, H2], BF, name=f"w04c_{name}", tag="w04c")
                b4c = wpool.tile([1, H2], BF, name=f"b4c_{name}", tag="b4c")
                nc.sync.dma_start(out=w04c[:], in_=w04_i[consumer])
                nc.sync.dma_start(out=b4c[:], in_=bmb4_i[consumer])
                _exp_w[name] = (w04c, b4c)

            def export_table(stateT, jlo, jhi, dst_dram, name):
                """rows j of dst = stateT_rows[j] @ (W0||W4) + (bmr||b4) of the
                consumer conv (pre-transformed gather table)."""
                w04c, b4c = _exp_w[name]
                for j in range(jlo, jhi):
                    jsl = slice(j * P, (j + 1) * P)
                    ptx = pe_p.tile([P, H2], FP, name=f"ptx_{name}_{j}", tag="pe")
                    for kh in range(2):
                        nc.tensor.matmul(ptx[:], stateT[:, kh, jsl],
                                         w04c[:, kh, :], start=kh == 0,
                                         stop=False, skip_group_check=True)
                    nc.tensor.matmul(ptx[:], ones1[:, :P], b4c[:],
                                     start=False, stop=True,
                                     skip_group_check=True)
                    rw = small.tile([P, H2], BF, name=f"rw_{name}_{j}", tag="rw")
                    nc.vector.tensor_copy(out=rw[:], in_=ptx[:])
                    nc.sync.dma_start(out=dst_dram[jsl, :], in_=rw[:])

            def export_final(stateT, dst_dram):
                for j in range(4):
                    jsl = slice(j * P, (j + 1) * P)
                    ptx = ptb_p.tile([P, 2, P], BF, name=f"ptf_{j}", tag="ptb")
                    for hf in range(2):
                        nc.tensor.transpose(ptx[:, hf, :], stateT[:, hf, jsl],
                                            ident[:])
                    rw = small.tile([P, H], FP, name=f"rwf_{j}", tag="rwf")
                    nc.vector.tensor_copy(
                        out=rw[:], in_=ptx[:].rearrange("p a b -> p (a b)"))
                    nc.sync.dma_start(out=dst_dram[jsl, :], in_=rw[:])

            # ---- network ----
            def ag_into(rows_ap, shape, name):
                tab_ = dram.tile(shape, BF, name=name, addr_space="Shared")
                nc.gpsimd.collective_compute(
                    "AllGather", mybir.AluOpType.bypass, replica_groups=RG,
                    ins=[rows_ap.opt()], outs=[tab_[:].opt()])
                return tab_[:]

            xtab = [xtab0_i[:]]

            def mk_post_p4(li, nxt, lab):
                mt_h = [None, None]
                JT = ECg // P

                def post_p4(ck):
                    # pipelined export: after chunk ck, j-tiles 8ck..8ck+7
                    # of yT are final
                    if ck == 0:
                        export_prep(li, f"m{lab}")
                    export_table(yT, 8 * ck, min(8 * (ck + 1), JT),
                                 m_rows, f"m{lab}")
                    if ck == CK_H0:
                        mt_h[0] = ag_into(m_rows[0:BHALF, :],
                                          [TABL // 2, H2], f"mtab{lab}h0")
                    if ck == CK_H1:
                        mt_h[1] = ag_into(m_rows[BHALF:, :],
                                          [TABL // 2, H2], f"mtab{lab}h1")
                        if nxt is not None:
                            export_prep(nxt, f"x{lab}")
                            export_table(xT, 0, 4, x_rows, f"x{lab}")
                            xtab[0] = ag_into(x_rows[:], [N_NODES, H2],
                                              f"xtab{lab}")
                return post_p4, mt_h

            for i in range(4):
                gi, li = 2 * i, 2 * i + 1
                nxt = 2 * i + 2 if i < 3 else 8
                post_p4, mt_h = mk_post_p4(li, nxt, str(i))
                emit_conv(gi, line=False, tabs=[xtab[0]], post_p4=post_p4)
                emit_conv(li, line=True, tabs=[mt_h[0], mt_h[1]])
            for i in range(8, 12):
                def post_3b(i=i):
                    export_prep(i + 1, f"x{i}")
                    export_table(xT, 0, 4, x_rows, f"x{i}")
                    xtab[0] = ag_into(x_rows[:], [N_NODES, H2], f"xtab{i}")
                emit_conv(i, line=False, tabs=[xtab[0]],
                          post_3b=post_3b if i < 11 else None)
            export_final(xT, out_o)

    nc.compile()
    return nc


# --------------------------------------------------------------------------
# entry point
# --------------------------------------------------------------------------

_CACHE = {}


def kernel(**inputs):
    import ml_dtypes
    from concourse import bass_utils
    bdt = ml_dtypes.bfloat16
    meta, in_maps = prep_all(inputs, bdt)
    key = (meta["gg"]["EC"], meta["gl"]["EC"],
           tuple(meta["gg"]["tile_win"]), tuple(meta["gl"]["tile_win"]))
    if key not in _CACHE:
        _CACHE.clear()
        _CACHE[key] = build_bass(meta)
    nc = _CACHE[key]
    res = bass_utils.run_bass_kernel_spmd(nc, in_maps,
                                          core_ids=list(range(NCORES)))
    outs = [r["out"] for r in res.results]
    return np.concatenate(outs, axis=0).astype(np.float32)



# revision 46
# speedup vs baseline: 1.0574x; 1.0574x over previous
"""ALIGNN (12x EdgeGatedGraphConv, H=256) on 8 TRN2 NeuronCores.

Sharding: nodes contiguously partitioned over cores; edges partitioned by dst
owner into 4 node-windows x 18 tiles (ECg=9216 slots/core), with a 2D-balanced
edge->tile assignment so each line-graph scatter window's per-gather-half load
fits 2 tiles. Line-graph gather table split in two position halves (crystal
windows {0,1} vs {2,3}); each half is exported + AllGather-ed as soon as the
covering phase-4 chunks finish, and line edge tiles are sorted half-0-first so
AG(h1) hides behind h0 tiles. Feature-major bf16 activations. Source-row
gathers via indirect DMA from row-major bf16 tables. e_dst expand + segment
sum via 0/1 indicator matmuls; m transposed to feature-major on PE. BN stats
via masked rank-1 matmuls + one small AllReduce per conv; BN affine + SiLU
fused on ScalarE.
"""
import numpy as np

H = 256
H2 = 512
NCORES = 8
P = 128
CHUNK = 1024
TPC = CHUNK // P  # tiles per chunk
N_NODES = 4096
N_EDGES = 49152
N_TRIPLETS = 262144
BN_EPS = 1e-5
F32 = np.float32
TILES_PER_WIN = 18  # crystal: diluted edge tiles per node-window
CAP = 2 * P         # per-(line-window, half) triplet budget -> 2 tiles
GATHER_BATCH = 1    # tiles per indirect gather (1 or 2)


# --------------------------------------------------------------------------
# host-side prep
# --------------------------------------------------------------------------

def _balance_buckets(src, dst, n_nodes, f0, f1):
    """Crystal graph: bucket edges by (dst owner, dst window); within each
    bucket assign edges to TILES_PER_WIN tiles of 128 slots, keeping each
    tile's (f0,f1) sums <= CAP where possible (controls line-graph padding).
    Returns per-core list of per-bucket edge-id lists (one list per tile)."""
    NR = n_nodes // NCORES
    NW = NR // P
    owner = dst // NR
    out = {}
    for c in range(NCORES):
        sel = np.where(owner == c)[0]
        w = (dst[sel] - c * NR) // P
        for wi in range(NW):
            ids = sel[w == wi]
            o = np.argsort(-(f0[ids] + f1[ids]), kind="stable")
            ids = ids[o]
            nt = TILES_PER_WIN
            l0 = np.zeros(nt); l1 = np.zeros(nt)
            cnt = np.zeros(nt, np.int64)
            tiles = [[] for _ in range(nt)]
            for e in ids:
                a, b = f0[e], f1[e]
                ok = (cnt < P) & (l0 + a <= CAP) & (l1 + b <= CAP)
                if ok.any():
                    cand = np.where(ok)[0]
                    k = cand[np.argmin(np.maximum(l0[cand] + a,
                                                  l1[cand] + b))]
                else:
                    cand = np.where(cnt < P)[0]
                    k = cand[np.argmin(np.maximum(l0[cand] + a,
                                                  l1[cand] + b))]
                tiles[k].append(e)
                l0[k] += a; l1[k] += b; cnt[k] += 1
            out[(c, wi)] = tiles
    return out


def _prep_crystal(src, dst, f0, f1):
    """Build crystal-graph layout with fixed TILES_PER_WIN tiles/window."""
    NR = N_NODES // NCORES
    NW = NR // P
    EC = NW * TILES_PER_WIN * P
    n_tiles = EC // P
    assn = _balance_buckets(src, dst, N_NODES, f0, f1)
    g = {"EC": EC, "NR": NR, "NW": NW, "NH": 1, "n_tiles": n_tiles,
         "tile_win": np.zeros(n_tiles, np.int64),
         "tile_half": np.zeros(n_tiles, np.int64),
         "tile_run_start": np.zeros(n_tiles, bool),
         "tile_run_end": np.zeros(n_tiles, bool),
         "tile_tail": np.zeros(n_tiles, bool),
         "idx": np.zeros((NCORES, EC), np.int32),
         "mask": np.zeros((NCORES, EC), F32),
         "ind_col": -np.ones((NCORES, EC), np.int64),
         "edge_id": -np.ones((NCORES, EC), np.int64)}
    t = 0
    for wi in range(NW):
        for k in range(TILES_PER_WIN):
            for c in range(NCORES):
                ids = np.array(assn[(c, wi)][k], np.int64)
                sl = slice(t * P, t * P + len(ids))
                g["idx"][c, sl] = src[ids].astype(np.int32)
                g["mask"][c, sl] = 1.0
                g["ind_col"][c, sl] = (dst[ids] - c * NR) - wi * P
                g["edge_id"][c, sl] = ids
            g["tile_win"][t] = wi
            g["tile_run_start"][t] = k == 0
            g["tile_run_end"][t] = k == TILES_PER_WIN - 1
            t += 1
    assert t == n_tiles and EC % CHUNK == 0
    return g


def _prep_line(src, dst, table_rows, pos_half, chunk=CHUNK):
    """Line graph: partition triplets by dst-slot owner; bucket by
    (gather half = src-slot position-half, dst window); pad per (half,window)
    to core-uniform tiles. idx values are relative to the half table."""
    E = len(src)
    NR = table_rows // NCORES
    NW = NR // P
    owner = dst // NR
    hs = pos_half // 2
    so, sp = src // pos_half, src % pos_half
    shalf = (sp >= hs).astype(np.int64)
    sidx = (so * hs + (sp % hs)).astype(np.int32)
    NH = 2
    buckets = {}
    for c in range(NCORES):
        sel = np.where(owner == c)[0]
        w = (dst[sel] - c * NR) // P
        hh = shalf[sel]
        for h in range(NH):
            for wi in range(NW):
                buckets[(c, h, wi)] = sel[(hh == h) & (w == wi)]
    T_hw = np.zeros((NH, NW), np.int64)
    for h in range(NH):
        for wi in range(NW):
            mx = max(len(buckets[(c, h, wi)]) for c in range(NCORES))
            T_hw[h, wi] = max(1, -(-mx // P))
    S_h = [int(T_hw[h].sum() * P) for h in range(NH)]
    S_h_pad = [-(-s // chunk) * chunk for s in S_h]
    EC = sum(S_h_pad)
    n_tiles = EC // P
    g = {"EC": EC, "NR": NR, "NW": NW, "NH": NH, "n_tiles": n_tiles,
         "tile_win": np.zeros(n_tiles, np.int64),
         "tile_half": np.zeros(n_tiles, np.int64),
         "tile_run_start": np.zeros(n_tiles, bool),
         "tile_run_end": np.zeros(n_tiles, bool),
         "tile_tail": np.zeros(n_tiles, bool),
         "idx": np.zeros((NCORES, EC), np.int32),
         "mask": np.zeros((NCORES, EC), F32),
         "ind_col": -np.ones((NCORES, EC), np.int64),
         "edge_id": -np.ones((NCORES, EC), np.int64)}
    t = pos = 0
    for h in range(NH):
        for wi in range(NW):
            nt = int(T_hw[h, wi])
            for c in range(NCORES):
                ids = buckets[(c, h, wi)]
                sl = slice(pos, pos + len(ids))
                g["idx"][c, sl] = sidx[ids]
                g["mask"][c, sl] = 1.0
                g["ind_col"][c, sl] = (dst[ids] - c * NR) - wi * P
                g["edge_id"][c, sl] = ids
            g["tile_win"][t:t + nt] = wi
            g["tile_half"][t:t + nt] = h
            g["tile_run_start"][t] = True
            g["tile_run_end"][t + nt - 1] = True
            t += nt
            pos += nt * P
        tail = (S_h_pad[h] - S_h[h]) // P
        for _ in range(tail):
            g["tile_win"][t] = 0
            g["tile_half"][t] = h
            g["tile_run_start"][t] = True
            g["tile_run_end"][t] = True
            g["tile_tail"][t] = True
            t += 1
            pos += P
    assert t == n_tiles and pos == EC
    return g


def _indicators(g, bdt):
    C, T = NCORES, g["n_tiles"]
    ind = np.zeros((C, T, P, P), F32)
    for c in range(C):
        cols = g["ind_col"][c]
        for t in range(T):
            cl = cols[t * P:(t + 1) * P]
            e = np.where(cl >= 0)[0]
            ind[c, t, e, cl[e]] = 1.0
    indT = ind.transpose(0, 1, 3, 2)
    # slab layout [T//TPC, 128, TPC, 128] (partition-major per chunk)
    def slab(a):
        return np.ascontiguousarray(
            a.reshape(C, T // TPC, TPC, P, P).transpose(0, 1, 3, 2, 4)).astype(bdt)
    return slab(ind), slab(indT)


def _cols(a, dt):  # [C, EC] -> [C, 128, T] (column t = tile t values)
    C, EC = a.shape
    return np.ascontiguousarray(
        a.reshape(C, EC // P, P).transpose(0, 2, 1)).astype(dt)


def _fm(a, bdt):  # [N, 256] -> [128, 2, N] feature-major
    return np.ascontiguousarray(a.T.reshape(2, P, -1).transpose(1, 0, 2)).astype(bdt)


def prep_all(inputs, bdt):
    src = np.asarray(inputs["src"]); dst = np.asarray(inputs["dst"])
    lsrc = np.asarray(inputs["lsrc"]); ldst = np.asarray(inputs["ldst"])
    # gather half of triplet t = crystal window-group of lsrc[t]'s dst node
    win_of_edge = (dst % (N_NODES // NCORES)) // P          # 0..3
    h_t = (win_of_edge[lsrc] >= 2).astype(np.int64)         # triplet half
    f0 = np.bincount(ldst[h_t == 0], minlength=N_EDGES)     # per-edge loads
    f1 = np.bincount(ldst[h_t == 1], minlength=N_EDGES)
    gg = _prep_crystal(src, dst, f0, f1)
    ECg = gg["EC"]
    BHALF = ECg // 2
    slot = np.zeros(N_EDGES, np.int64)
    for c in range(NCORES):
        eids = gg["edge_id"][c]
        r = eids >= 0
        slot[eids[r]] = c * ECg + np.where(r)[0]
    TABL = NCORES * ECg
    gl = _prep_line(slot[lsrc], slot[ldst], TABL, ECg)
    meta = {"gg": gg, "gl": gl, "TABL": TABL}

    x = np.asarray(inputs["x"]); y = np.asarray(inputs["y"])
    z = np.asarray(inputs["z"])
    W = np.asarray(inputs["W"]); b = np.asarray(inputs["b"])
    bg = np.asarray(inputs["bn_gamma"]); bb = np.asarray(inputs["bn_beta"])
    ECl = gl["EC"]

    ind_g, indT_g = _indicators(gg, bdt)
    ind_l, indT_l = _indicators(gl, bdt)
    mask_g = _cols(gg["mask"], bdt)
    mask_l = _cols(gl["mask"], bdt)
    idx_g = _cols(gg["idx"].astype(np.int32), np.int32)
    idx_l = _cols(gl["idx"].astype(np.int32), np.int32)

    # per-core feature tensors
    xT = np.zeros((NCORES, P, 2, 512), bdt)
    yT = np.zeros((NCORES, P, 2, ECg), bdt)
    zT = np.zeros((NCORES, P, 2, ECl), bdt)
    for c in range(NCORES):
        xT[c] = _fm(x[c * 512:(c + 1) * 512], bdt)
        eg = gg["edge_id"][c]; r = eg >= 0
        yr = np.zeros((ECg, H), F32); yr[r] = y[eg[r]]
        yT[c] = _fm(yr, bdt)
        el = gl["edge_id"][c]; rl = el >= 0
        zr = np.zeros((ECl, H), F32); zr[rl] = z[el[rl]]
        zT[c] = _fm(zr, bdt)
    mbcg = np.repeat(gg["mask"][:, None, :], P, axis=1).astype(bdt)  # [C,128,ECg]
    xb = x.astype(bdt).astype(F32)
    W0b = W[0, 0].astype(bdt).astype(F32)
    W4b = W[0, 4].astype(bdt).astype(F32)
    b4b = b[0, 4].astype(F32)
    bm0 = (b[0, 0] + b[0, 1] + b[0, 2]).astype(F32)
    xtab0 = np.concatenate([xb @ W0b + bm0, xb @ W4b + b4b], 1).astype(bdt)

    # weights: w04 [12,128,2,512]; w1/w2 [12,128,2,256]; w3 [12,128,2,2,128]
    w04 = np.concatenate([W[:, 0], W[:, 4]], axis=2)  # [12,256,512]
    w04 = w04.reshape(12, 2, P, H2).transpose(0, 2, 1, 3).astype(bdt).copy()
    w1 = W[:, 1].reshape(12, 2, P, H).transpose(0, 2, 1, 3).astype(bdt).copy()
    w2 = W[:, 2].reshape(12, 2, P, H).transpose(0, 2, 1, 3).astype(bdt).copy()
    w3 = W[:, 3].reshape(12, 2, P, 2, P).transpose(0, 2, 1, 3, 4).astype(bdt).copy()
    bmr = (b[:, 0] + b[:, 1] + b[:, 2]).reshape(12, 1, H)
    b4r = b[:, 4].reshape(12, 1, H)
    bmb4 = np.concatenate([bmr, b4r], axis=2).astype(bdt)  # [12,1,512]
    b3r = b[:, 3].reshape(12, 1, 2, P).astype(bdt)
    # gamma/beta packed [12,128,4]: cols (node h0, node h1, edge h0, edge h1)
    gbg = np.stack([bg[:, 0, :P], bg[:, 0, P:], bg[:, 1, :P], bg[:, 1, P:]], -1)
    gbb = np.stack([bb[:, 0, :P], bb[:, 0, P:], bb[:, 1, :P], bb[:, 1, P:]], -1)

    # pad corrections for unmasked node stats of line convs: pad slots hold
    # exactly b3(bf16) after the node update; subtract their sums post-AR.
    npads = float(NCORES * ECg - N_EDGES)
    b3bf = b[:, 3].astype(bdt).astype(F32)               # [12, 256]
    scorr = np.zeros((12, P, 2), F32)
    qcorr = np.zeros((12, P, 2), F32)
    for li in (1, 3, 5, 7):
        v = b3bf[li].reshape(2, P).T                     # [P, 2]
        scorr[li] = npads * v
        qcorr[li] = npads * v * v

    shared = {"xtab0": xtab0, "w04": w04, "w1": w1, "w2": w2, "w3": w3,
              "bmb4": bmb4, "b3r": b3r,
              "gbg": gbg.astype(F32), "gbb": gbb.astype(F32),
              "scorr": scorr, "qcorr": qcorr}
    in_maps = []
    for c in range(NCORES):
        m = dict(shared)
        m.update({"xT": xT[c], "yT": yT[c], "zT": zT[c],
                  "idxg": idx_g[c], "idxl": idx_l[c],
                  "indg": ind_g[c], "indTg": indT_g[c], "maskg": mask_g[c],
                  "indl": ind_l[c], "indTl": indT_l[c], "maskl": mask_l[c],
                  "mbcg": mbcg[c]})
        in_maps.append(m)
    return meta, in_maps


# --------------------------------------------------------------------------
# bass kernel
# --------------------------------------------------------------------------

def build_bass(meta):
    import concourse.bass as bass
    import concourse.bacc as bacc
    import concourse.tile as tile
    from concourse import mybir
    from concourse.masks import make_identity

    BF = mybir.dt.bfloat16
    FP = mybir.dt.float32
    I32 = mybir.dt.int32
    AL = mybir.AluOpType
    AF = mybir.ActivationFunctionType
    AX = mybir.AxisListType
    RG = [list(range(NCORES))]

    gg, gl = meta["gg"], meta["gl"]
    ECg, ECl = gg["EC"], gl["EC"]
    NWg, NWl = gg["NW"], gl["NW"]
    Tg, Tl = gg["n_tiles"], gl["n_tiles"]
    TABL = meta["TABL"]
    BHALF = ECg // 2          # line gather-table half boundary (positions)
    JH = BHALF // P           # export j-tiles per half
    CK_H0 = -(-BHALF // CHUNK) - 1   # phase-4 chunk covering half 0
    CK_H1 = ECg // CHUNK - 1

    nc = bacc.Bacc("TRN2", target_bir_lowering=False, debug=False,
                   num_devices=NCORES)

    def din(name, shape, dt):
        return nc.dram_tensor(name, shape, dt, kind="ExternalInput").ap()

    xT_i = din("xT", [P, 2, 512], BF)
    yT_i = din("yT", [P, 2, ECg], BF)
    zT_i = din("zT", [P, 2, ECl], BF)
    xtab0_i = din("xtab0", [N_NODES, H2], BF)
    idxg_i = din("idxg", [P, Tg], I32)
    idxl_i = din("idxl", [P, Tl], I32)
    indg_i = din("indg", [Tg // TPC, P, TPC, P], BF)
    indTg_i = din("indTg", [Tg // TPC, P, TPC, P], BF)
    maskg_i = din("maskg", [P, Tg], BF)
    indl_i = din("indl", [Tl // TPC, P, TPC, P], BF)
    indTl_i = din("indTl", [Tl // TPC, P, TPC, P], BF)
    maskl_i = din("maskl", [P, Tl], BF)
    mbcg_i = din("mbcg", [P, ECg], BF)
    w04_i = din("w04", [12, P, 2, H2], BF)
    w1_i = din("w1", [12, P, 2, H], BF)
    w2_i = din("w2", [12, P, 2, H], BF)
    w3_i = din("w3", [12, P, 2, 2, P], BF)
    bmb4_i = din("bmb4", [12, 1, H2], BF)
    b3r_i = din("b3r", [12, 1, 2, P], BF)
    gbg_i = din("gbg", [12, P, 4], FP)
    gbb_i = din("gbb", [12, P, 4], FP)
    scorr_i = din("scorr", [12, P, 2], FP)
    qcorr_i = din("qcorr", [12, P, 2], FP)
    out_o = nc.dram_tensor("out", [512, H], FP, kind="ExternalOutput").ap()

    with tile.TileContext(nc) as tc:
        with (
            tc.tile_pool(name="dram", bufs=1, space="DRAM") as dram,
            tc.tile_pool(name="dram2", bufs=2, space="DRAM") as dram2,
            tc.tile_pool(name="persist", bufs=1) as persist,
            tc.tile_pool(name="wpool", bufs=2) as wpool,
            tc.tile_pool(name="chk", bufs=2) as chk,
            tc.tile_pool(name="work", bufs=3) as work,
            tc.tile_pool(name="small", bufs=2) as small,
            tc.tile_pool(name="pe", bufs=2, space="PSUM") as pe_p,
            tc.tile_pool(name="pm", bufs=2, space="PSUM") as pm_p,
            tc.tile_pool(name="pn", bufs=2, space="PSUM") as pn_p,
            tc.tile_pool(name="ptb", bufs=1, space="PSUM") as ptb_p,
            tc.tile_pool(name="ps", bufs=1, space="PSUM") as ps_p,
        ):
            z_state = dram.tile([P, 2, ECl], BF)

            m_rows = dram.tile([ECg, H2], BF)
            x_rows = dram.tile([512, H2], BF)

            yT = persist.tile([P, 2, ECg], BF)
            xT = persist.tile([P, 2, 512], BF)
            mbcg = persist.tile([P, ECg], BF)
            edst_l = persist.tile([P, NWl, H], BF)
            edst_g = persist.tile([P, NWg, H], BF)
            idxg = persist.tile([P, Tg], I32)
            idxl = persist.tile([P, Tl], I32)
            maskg_t = persist.tile([P, Tg], BF)
            maskl_t = persist.tile([P, Tl], BF)
            ident = persist.tile([P, P], BF)
            identF = persist.tile([P, P], FP)
            ones1 = persist.tile([1, H2], BF)
            onesP = persist.tile([P, 1], BF)
            epsb = persist.tile([P, 1], FP)

            nc.sync.dma_start(out=yT[:], in_=yT_i[:])
            nc.sync.dma_start(out=xT[:], in_=xT_i[:])
            nc.sync.dma_start(out=mbcg[:], in_=mbcg_i[:])
            nc.sync.dma_start(out=idxg[:], in_=idxg_i[:])
            nc.sync.dma_start(out=idxl[:], in_=idxl_i[:])
            nc.sync.dma_start(out=maskg_t[:], in_=maskg_i[:])
            nc.sync.dma_start(out=maskl_t[:], in_=maskl_i[:])
            make_identity(nc, ident[:])
            make_identity(nc, identF[:])
            nc.gpsimd.memset(ones1[:], 1.0)
            nc.gpsimd.memset(onesP[:], 1.0)
            nc.gpsimd.memset(epsb[:], 1e-6)

            def emit_conv(i, line, tabs, post_3b=None, post_p4=None):
                g = gl if line else gg
                T, NW, EC = g["n_tiles"], g["NW"], g["EC"]
                NH = g["NH"]
                n_rn = N_EDGES if line else N_NODES
                n_re = N_TRIPLETS if line else N_EDGES
                nfT = yT if line else xT
                edst = edst_l if line else edst_g
                idx = idxl if line else idxg
                ind_i, indT_i, mask_i = ((indl_i, indTl_i, maskl_i) if line
                                         else (indg_i, indTg_i, maskg_i))

                w1 = wpool.tile([P, 2, H], BF, tag="w1")
                w2 = wpool.tile([P, 2, H], BF, tag="w2")
                w3 = wpool.tile([P, 2, 2, P], BF, tag="w3")
                b3r = wpool.tile([1, 2, P], BF, tag="b3r")
                gbg = wpool.tile([P, 4], FP, tag="gbg")
                gbb = wpool.tile([P, 4], FP, tag="gbb")
                invn = wpool.tile([P, 4], FP, tag="invn")
                scor = wpool.tile([P, 2], FP, tag="scor")
                qcor = wpool.tile([P, 2], FP, tag="qcor")
                for tl, src_t in ((w1, w1_i), (w2, w2_i),
                                  (w3, w3_i),
                                  (b3r, b3r_i), (gbg, gbg_i), (gbb, gbb_i),
                                  (scor, scorr_i), (qcor, qcorr_i)):
                    nc.gpsimd.dma_start(out=tl[:], in_=src_t[i])
                nc.vector.memset(invn[:, 0:2], 1.0 / n_rn)
                nc.vector.memset(invn[:, 2:4], 1.0 / n_re)

                # ---- node phase: e_dst windows (bias folded into tables) ----
                for w in range(NW):
                    pw = pe_p.tile([P, H2], FP, tag="pe")
                    sl = slice(w * P, (w + 1) * P)
                    for kh in range(2):
                        nc.tensor.matmul(pw[:, :H], nfT[:, kh, sl], w1[:, kh, :],
                                         start=kh == 0, stop=kh == 1,
                                         skip_group_check=True)
                    nc.scalar.activation(edst[:, w, :], pw[:, :H], AF.Copy)

                # ---- edge phase (3a interleaved per completed window) ----
                m_scr = dram2.tile([P, 2, ECl], BF, tag="mscr",
                                   name=f"mscr_{i}")
                naccl_d = None
                if line:
                    naccl_d = dram2.tile([NWl, P, 4 * P], BF, tag="naccl",
                                         name=f"naccl_{i}")
                pstat = ps_p.tile([1, H2], FP, tag="ps")
                pnode = None
                mask_t = maskl_t if line else maskg_t
                scs = small.tile([P, NW, 4], FP, name=f"scs_{i}", tag="scs")

                def emit_3a(w, pn_ap):
                    """Node update for window w; pn_ap = [P, 512] accumulated
                    (sum_sigma_h || sum_sigma), PSUM or SBUF."""
                    wsl = slice(w * P, (w + 1) * P)
                    den = small.tile([P, H], FP, tag="den")
                    hrow = small.tile([P, H], FP, tag="hrow")
                    nc.vector.tensor_scalar_add(den[:], pn_ap[:, H:], 1e-6)
                    nc.vector.reciprocal(den[:], den[:])
                    nc.vector.tensor_tensor(out=hrow[:], in0=pn_ap[:, :H],
                                            in1=den[:], op=AL.mult)
                    pnu = pe_p.tile([P, H2], FP, tag="pe")
                    for b_ in range(2):
                        bsl = slice(b_ * P, (b_ + 1) * P)
                        for a_ in range(2):
                            nc.tensor.matmul(pnu[:, bsl], w3[:, a_, b_, :],
                                             nfT[:, a_, wsl], start=a_ == 0,
                                             stop=False, skip_group_check=True)
                        nc.tensor.matmul(pnu[:, bsl], b3r[:, b_, :],
                                         ones1[:, :P], start=False, stop=False,
                                         skip_group_check=True)
                        # h^T accumulated straight into the psum group
                        nc.tensor.matmul(pnu[:, bsl], hrow[:, bsl], identF[:],
                                         is_transpose=True, start=False,
                                         stop=b_ == 1, skip_group_check=True)
                    # store nodupd + BN stats (unmasked; host pad-corrections)
                    nup = edst[:, w, :]
                    nc.scalar.activation(nup, pnu[:, :H], AF.Copy)
                    tq = small.tile([P, 2, P], BF, tag="tq")
                    nup2 = nup.rearrange("p (a b) -> p a b", a=2)
                    nc.scalar.square(tq[:], nup2)
                    nc.vector.tensor_reduce(scs[:, w, 0:2], nup2,
                                            axis=AX.X, op=AL.add)
                    nc.vector.tensor_reduce(scs[:, w, 2:4], tq[:],
                                            axis=AX.X, op=AL.add)

                for ck in range(EC // CHUNK):
                    t0 = ck * TPC
                    csl = slice(ck * CHUNK, (ck + 1) * CHUNK)
                    if line:
                        zt = chk.tile([P, 2, CHUNK], BF, tag="zt")
                        zsrc = zT_i if i == 1 else z_state
                        nc.sync.dma_start(out=zt[:], in_=zsrc[:, :, csl])
                    inds = chk.tile([P, TPC, P], BF, tag="inds")
                    indTs = chk.tile([P, TPC, P], BF, tag="indTs")
                    mchk = chk.tile([P, 2, CHUNK], BF, tag="mchk")
                    nc.sync.dma_start(out=inds[:], in_=ind_i[ck])
                    nc.sync.dma_start(out=indTs[:], in_=indT_i[ck])

                    gath = None
                    sbm = None
                    ptb4 = None
                    for tt in range(TPC):
                        t = t0 + tt
                        w = int(g["tile_win"][t])
                        hh = int(g["tile_half"][t])
                        tail = bool(g["tile_tail"][t])
                        # gathers batched in pairs when same table half
                        gq = tt % 2
                        if gq == 0:
                            pair = (GATHER_BATCH == 2
                                    and tt + 1 < TPC
                                    and not tail
                                    and not bool(g["tile_tail"][t + 1])
                                    and int(g["tile_half"][t + 1]) == hh)
                            gath = work.tile([P, 2, H2], BF, tag="gath", bufs=4)
                            if pair:
                                nc.gpsimd.indirect_dma_start(
                                    out=gath[:], out_offset=None, in_=tabs[hh],
                                    in_offset=bass.IndirectOffsetOnAxis(
                                        ap=idx[:, t:t + 2], axis=0))
                        if not (gq == 0 and pair):
                            if tail:
                                nc.vector.memset(gath[:, gq, :], 0.0)
                            elif gq == 1 and not pair:
                                nc.gpsimd.indirect_dma_start(
                                    out=gath[:, 1, :], out_offset=None,
                                    in_=tabs[hh],
                                    in_offset=bass.IndirectOffsetOnAxis(
                                        ap=idx[:, t:t + 1], axis=0))
                            elif gq == 0:
                                nc.gpsimd.indirect_dma_start(
                                    out=gath[:, 0, :], out_offset=None,
                                    in_=tabs[hh],
                                    in_offset=bass.IndirectOffsetOnAxis(
                                        ap=idx[:, t:t + 1], axis=0))
                        ga = gath[:, gq, :]
                        pm = pm_p.tile([P, H], FP, tag="pm")
                        nc.tensor.matmul(pm[:], indTs[:, tt, :],
                                         edst[:, w, :], start=True, stop=False)
                        for kh in range(2):
                            ef = (zt[:, kh, tt * P:(tt + 1) * P] if line
                                  else yT[:, kh, ck * CHUNK + tt * P:
                                          ck * CHUNK + (tt + 1) * P])
                            nc.tensor.matmul(pm[:], ef, w2[:, kh, :],
                                             start=False, stop=kh == 1)
                        if gq == 0:
                            sbm = work.tile([P, 2, H2], BF, tag="sbm", bufs=3)
                        sbw = work.tile([P, H2], BF, tag="sbw", bufs=6)
                        # m = e_src(gathered, +bias) + e_dst_expand + egate
                        nc.vector.tensor_tensor(out=sbm[:, gq, :H],
                                                in0=ga[:, :H],
                                                in1=pm[:], op=AL.add)
                        nc.scalar.activation(sbw[:, H:], sbm[:, gq, :H],
                                             AF.Sigmoid)
                        nc.scalar.square(sbm[:, gq, H:], sbm[:, gq, :H])
                        nc.vector.tensor_tensor(out=sbw[:, :H], in0=sbw[:, H:],
                                                in1=ga[:, H:], op=AL.mult)
                        if g["tile_run_start"][t]:
                            pnode = pn_p.tile([P, 4 * P], FP, tag="pn")
                        nc.tensor.matmul(pnode[:], inds[:, tt, :], sbw[:],
                                         start=bool(g["tile_run_start"][t]),
                                         stop=bool(g["tile_run_end"][t]),
                                         skip_group_check=True)
                        nc.tensor.matmul(pstat[:], mask_t[:, t:t + 1],
                                         sbm[:, gq, :],
                                         start=t == 0, stop=t == T - 1,
                                         skip_group_check=True)
                        # m transposed to feature-major, 4 tiles per psum tile
                        q4 = tt % 4
                        if q4 == 0:
                            ptb4 = ptb_p.tile([P, 2, 4, P], BF, tag="ptb")
                        nc.tensor.transpose(ptb4[:, 0, q4, :],
                                            sbm[:, gq, :P], ident[:])
                        nc.tensor.transpose(ptb4[:, 1, q4, :],
                                            sbm[:, gq, P:H], ident[:])
                        if q4 == 3:
                            gsl = slice((tt - 3) * P, (tt + 1) * P)
                            nc.vector.tensor_copy(
                                out=mchk[:, :, gsl], in_=ptb4[:])
                        if tt == TPC - 1:
                            nc.sync.dma_start(out=m_scr[:, :, csl], in_=mchk[:])
                        if g["tile_run_end"][t] and not tail:
                            if line and hh == 0:
                                nae = small.tile([P, 4 * P], BF, tag="nae")
                                nc.vector.tensor_copy(out=nae[:], in_=pnode[:])
                                nc.sync.dma_start(out=naccl_d[w], in_=nae[:])
                            elif line:
                                na0 = small.tile([P, 4 * P], BF, tag="naccld")
                                nc.sync.dma_start(out=na0[:], in_=naccl_d[w])
                                nacc = small.tile([P, 4 * P], FP, tag="naccs")
                                nc.vector.tensor_tensor(out=nacc[:], in0=na0[:],
                                                        in1=pnode[:], op=AL.add)
                                emit_3a(w, nacc)
                            else:
                                emit_3a(w, pnode)

                # ---- stats allreduce ----
                nst = small.tile([P, 4], FP, tag="nst")
                for j in range(4):
                    nc.vector.tensor_reduce(nst[:, j:j + 1], scs[:, :, j],
                                            axis=AX.X, op=AL.add)
                est = small.tile([1, H2], FP, tag="est")
                nc.vector.tensor_copy(out=est[:], in_=pstat[:])
                stb_i = dram2.tile([1024], FP, tag="stbi")
                stb_o = dram2.tile([1024], FP, tag="stbo")
                nc.sync.dma_start(
                    out=stb_i[0:H2].rearrange("(c p) -> p c", p=P), in_=nst[:])
                nc.sync.dma_start(out=stb_i[H2:].rearrange("(a b) -> a b", a=1), in_=est[:])
                nc.gpsimd.collective_compute(
                    "AllReduce", AL.add, replica_groups=RG,
                    ins=[stb_i[:].opt()], outs=[stb_o[:].opt()])
                SS = small.tile([P, 4], FP, tag="SS")
                QQ = small.tile([P, 4], FP, tag="QQ")
                nc.sync.dma_start(out=SS[:, 0:2],
                                  in_=stb_o[0:256].rearrange("(c p) -> p c", p=P))
                nc.sync.dma_start(out=QQ[:, 0:2],
                                  in_=stb_o[256:512].rearrange("(c p) -> p c", p=P))
                nc.sync.dma_start(out=SS[:, 2:4],
                                  in_=stb_o[512:768].rearrange("(c p) -> p c", p=P))
                nc.sync.dma_start(out=QQ[:, 2:4],
                                  in_=stb_o[768:1024].rearrange("(c p) -> p c", p=P))
                # pad-slot corrections for the unmasked node stats
                nc.vector.tensor_tensor(out=SS[:, 0:2], in0=SS[:, 0:2],
                                        in1=scor[:], op=AL.subtract)
                nc.vector.tensor_tensor(out=QQ[:, 0:2], in0=QQ[:, 0:2],
                                        in1=qcor[:], op=AL.subtract)
                mu = small.tile([P, 4], FP, tag="mu")
                var = small.tile([P, 4], FP, tag="var")
                sc4 = small.tile([P, 4], FP, tag="sc4")
                sh4 = small.tile([P, 4], FP, tag="sh4")
                nc.vector.tensor_tensor(out=mu[:], in0=SS[:], in1=invn[:], op=AL.mult)
                nc.vector.tensor_tensor(out=var[:], in0=QQ[:], in1=invn[:], op=AL.mult)
                nc.vector.tensor_tensor(out=sc4[:], in0=mu[:], in1=mu[:], op=AL.mult)
                nc.vector.tensor_tensor(out=var[:], in0=var[:], in1=sc4[:],
                                        op=AL.subtract)
                nc.vector.tensor_scalar_add(var[:], var[:], BN_EPS)
                nc.scalar.sqrt(var[:], var[:])
                nc.vector.reciprocal(var[:], var[:])
                nc.vector.tensor_tensor(out=sc4[:], in0=var[:], in1=gbg[:], op=AL.mult)
                nc.vector.tensor_tensor(out=mu[:], in0=mu[:], in1=sc4[:], op=AL.mult)
                nc.vector.tensor_tensor(out=sh4[:], in0=gbb[:], in1=mu[:],
                                        op=AL.subtract)

                # ---- phase 3b: node update ----
                for w in range(NW):
                    wsl = slice(w * P, (w + 1) * P)
                    t3 = small.tile([P, 2, P], BF, tag="t3")
                    for hf in range(2):
                        nc.scalar.activation(t3[:, hf, :],
                                             edst[:, w, hf * P:(hf + 1) * P],
                                             AF.Silu, bias=sh4[:, hf:hf + 1],
                                             scale=sc4[:, hf:hf + 1])
                        if line:
                            nc.vector.tensor_tensor(out=t3[:, hf, :],
                                                    in0=t3[:, hf, :],
                                                    in1=mbcg[:, wsl], op=AL.mult)
                    nc.vector.tensor_tensor(out=nfT[:, :, wsl],
                                            in0=nfT[:, :, wsl], in1=t3[:],
                                            op=AL.add)
                if post_3b is not None:
                    post_3b()

                # ---- phase 4: edge update ----
                for ck in range(EC // CHUNK):
                    csl = slice(ck * CHUNK, (ck + 1) * CHUNK)
                    mt = chk.tile([P, 2, CHUNK], BF, tag="mt")
                    nc.sync.dma_start(out=mt[:], in_=m_scr[:, :, csl])
                    if line:
                        zt2 = chk.tile([P, 2, CHUNK], BF, tag="zt")
                        zsrc = zT_i if i == 1 else z_state
                        nc.sync.dma_start(out=zt2[:], in_=zsrc[:, :, csl])
                    for hf in range(2):
                        nc.scalar.activation(mt[:, hf, :], mt[:, hf, :],
                                             AF.Silu,
                                             bias=sh4[:, 2 + hf:3 + hf],
                                             scale=sc4[:, 2 + hf:3 + hf])
                    if line:
                        for hf in range(2):
                            nc.vector.tensor_tensor(out=mt[:, hf, :],
                                                    in0=mt[:, hf, :],
                                                    in1=zt2[:, hf, :],
                                                    op=AL.add)
                        nc.sync.dma_start(out=z_state[:, :, csl],
                                          in_=mt[:])
                    else:
                        for hf in range(2):
                            nc.vector.tensor_tensor(out=mt[:, hf, :],
                                                    in0=mt[:, hf, :],
                                                    in1=mbcg[:, csl], op=AL.mult)
                            nc.vector.tensor_tensor(out=yT[:, hf, csl],
                                                    in0=yT[:, hf, csl],
                                                    in1=mt[:, hf, :],
                                                    op=AL.add)
                    if post_p4 is not None:
                        post_p4(ck)

            # ---- table exports ----
            _exp_w = {}

            def export_prep(consumer, name):
                w04c = wpool.tile([P, 2, H2], BF, name=f"w04c_{name}", tag="w04c")
                b4c = wpool.tile([1, H2], BF, name=f"b4c_{name}", tag="b4c")
                nc.sync.dma_start(out=w04c[:], in_=w04_i[consumer])
                nc.sync.dma_start(out=b4c[:], in_=bmb4_i[consumer])
                _exp_w[name] = (w04c, b4c)

            def export_table(stateT, jlo, jhi, dst_dram, name):
                """rows j of dst = stateT_rows[j] @ (W0||W4) + (bmr||b4) of the
                consumer conv (pre-transformed gather table)."""
                w04c, b4c = _exp_w[name]
                for j in range(jlo, jhi):
                    jsl = slice(j * P, (j + 1) * P)
                    ptx = pe_p.tile([P, H2], FP, name=f"ptx_{name}_{j}", tag="pe")
                    for kh in range(2):
                        nc.tensor.matmul(ptx[:], stateT[:, kh, jsl],
                                         w04c[:, kh, :], start=kh == 0,
                                         stop=False, skip_group_check=True)
                    nc.tensor.matmul(ptx[:], ones1[:, :P], b4c[:],
                                     start=False, stop=True,
                                     skip_group_check=True)
                    rw = small.tile([P, H2], BF, name=f"rw_{name}_{j}", tag="rw")
                    nc.vector.tensor_copy(out=rw[:], in_=ptx[:])
                    nc.sync.dma_start(out=dst_dram[jsl, :], in_=rw[:])

            def export_final(stateT, dst_dram):
                for j in range(4):
                    jsl = slice(j * P, (j + 1) * P)
                    ptx = ptb_p.tile([P, 2, P], BF, name=f"ptf_{j}", tag="ptb")
                    for hf in range(2):
                        nc.tensor.transpose(ptx[:, hf, :], stateT[:, hf, jsl],
                                            ident[:])
                    rw = small.tile([P, H], FP, name=f"rwf_{j}", tag="rwf")
                    nc.vector.tensor_copy(
                        out=rw[:], in_=ptx[:].rearrange("p a b -> p (a b)"))
                    nc.sync.dma_start(out=dst_dram[jsl, :], in_=rw[:])

            # ---- network ----
            def ag_into(rows_ap, shape, name):
                tab_ = dram.tile(shape, BF, name=name, addr_space="Shared")
                nc.gpsimd.collective_compute(
                    "AllGather", mybir.AluOpType.bypass, replica_groups=RG,
                    ins=[rows_ap.opt()], outs=[tab_[:].opt()])
                return tab_[:]

            xtab = [xtab0_i[:]]

            def mk_post_p4(li, nxt, lab):
                mt_h = [None, None]
                JT = ECg // P

                def post_p4(ck):
                    # pipelined export: after chunk ck, j-tiles 8ck..8ck+7
                    # of yT are final
                    if ck == 0:
                        export_prep(li, f"m{lab}")
                    export_table(yT, 8 * ck, min(8 * (ck + 1), JT),
                                 m_rows, f"m{lab}")
                    if ck == CK_H0:
                        mt_h[0] = ag_into(m_rows[0:BHALF, :],
                                          [TABL // 2, H2], f"mtab{lab}h0")
                    if ck == CK_H1:
                        mt_h[1] = ag_into(m_rows[BHALF:, :],
                                          [TABL // 2, H2], f"mtab{lab}h1")
                        if nxt is not None:
                            export_prep(nxt, f"x{lab}")
                            export_table(xT, 0, 4, x_rows, f"x{lab}")
                            xtab[0] = ag_into(x_rows[:], [N_NODES, H2],
                                              f"xtab{lab}")
                return post_p4, mt_h

            for i in range(4):
                gi, li = 2 * i, 2 * i + 1
                nxt = 2 * i + 2 if i < 3 else 8
                post_p4, mt_h = mk_post_p4(li, nxt, str(i))
                emit_conv(gi, line=False, tabs=[xtab[0]], post_p4=post_p4)
                emit_conv(li, line=True, tabs=[mt_h[0], mt_h[1]])
            for i in range(8, 12):
                def post_3b(i=i):
                    export_prep(i + 1, f"x{i}")
                    export_table(xT, 0, 4, x_rows, f"x{i}")
                    xtab[0] = ag_into(x_rows[:], [N_NODES, H2], f"xtab{i}")
                emit_conv(i, line=False, tabs=[xtab[0]],
                          post_3b=post_3b if i < 11 else None)
            export_final(xT, out_o)

    nc.compile()
    return nc


# --------------------------------------------------------------------------
# entry point
# --------------------------------------------------------------------------

_CACHE = {}


def kernel(**inputs):
    import ml_dtypes
    from concourse import bass_utils
    bdt = ml_dtypes.bfloat16
    meta, in_maps = prep_all(inputs, bdt)
    key = (meta["gg"]["EC"], meta["gl"]["EC"],
           tuple(meta["gg"]["tile_win"]), tuple(meta["gl"]["tile_win"]))
    if key not in _CACHE:
        _CACHE.clear()
        _CACHE[key] = build_bass(meta)
    nc = _CACHE[key]
    res = bass_utils.run_bass_kernel_spmd(nc, in_maps,
                                          core_ids=list(range(NCORES)))
    outs = [r["out"] for r in res.results]
    return np.concatenate(outs, axis=0).astype(np.float32)

